# revision 1
# baseline (speedup 1.0000x reference)
"""Trainium2 Bass kernel for nn_BidirRecurrentModel (2-layer bidirectional GRU).

Strategy notes
--------------
The model output depends only on:
  * seq_f[-1] after layer-2 forward  -> needs the full 2x128-step forward recurrence
  * seq_r[0]  after layer-2 reverse  -> needs only ONE step per layer of the
    reverse cells (h0 = 0), fed by x[:, T-1, :]
so almost all work is the forward recurrence, which is latency-bound.

Layouts (hardcoded for B=32, T=128, I=H=O=512):
  packed [128, 128] tile: row 32*k + b <-> (chunk k of 4, batch b), col j = dim-in-chunk
  hT     [128, 128] bf16: hT[p, 32*k+b] = h[b, 128*k+p]  (stationary operand slices)
PSUM gate bank [128, 512] = [u | r | z | xr] accumulates:
  identity-matmul(bias consts) + x/xh-projection stream + h-recurrence stream,
  each as 4-way column-tiled matmuls (batch=32 per column group).
All matmul inputs bf16 (fp32 accumulate in PSUM); elementwise chain in fp32.
"""

import numpy as np

import concourse.bass as bass
import concourse.mybir as mybir
import concourse.tile as tile
from concourse import bacc
from concourse.bass_utils import run_bass_kernel_spmd

F32 = mybir.dt.float32
BF16 = mybir.dt.bfloat16
AF = mybir.ActivationFunctionType

B, T, I, H, O = 32, 128, 512, 512, 512
KC = 4          # 128-chunks over H (and I)
NCORES = 8

import ml_dtypes
BFNP = ml_dtypes.bfloat16


def _to_bf16(a):
    return np.asarray(a, np.float32).astype(BFNP)


def _pack_vec(v):
    """[512] -> packed replicated [128, 128]: out[32k+b, j] = v[128k+j]."""
    v = np.asarray(v, np.float32).reshape(KC, 128)
    out = np.repeat(v[:, None, :], B, axis=1)
    return out.reshape(128, 128)


def _gate_mov_tiles(Whh, Whr):
    """Moving tiles for the h-recurrence stream, PSUM bank layout [u|r|z|xr].

    Whh: [512, 1024] (z cols 0:512, r cols 512:1024), Whr: [512, 512].
    Returns [128, KC*4*384]: free index (k, gc, [u|r|z] x 128) -> bank 0:384.
    """
    Wz = Whh[:, :H]
    Wr = Whh[:, H:]
    out = np.zeros((128, KC * 4 * 384), np.float32)
    for k in range(KC):
        rows = slice(128 * k, 128 * (k + 1))
        for gc in range(4):
            cols = slice(128 * gc, 128 * (gc + 1))
            base = (k * 4 + gc) * 384
            out[:, base : base + 128] = Whr[rows, cols]
            out[:, base + 128 : base + 256] = Wr[rows, cols]
            out[:, base + 256 : base + 384] = Wz[rows, cols]
    return _to_bf16(out)


def _x_mov_tiles(Wxh, Wxr):
    """Moving tiles for the input-projection stream -> bank 128:512 [r|z|xr].

    Returns [128, KC*4*384]."""
    Wz = Wxh[:, :H]
    Wr = Wxh[:, H:]
    rz = np.zeros((128, KC * 4 * 384), np.float32)
    for k in range(KC):
        rows = slice(128 * k, 128 * (k + 1))
        for gc in range(4):
            cols = slice(128 * gc, 128 * (gc + 1))
            base = (k * 4 + gc) * 384
            rz[:, base : base + 128] = Wr[rows, cols]
            rz[:, base + 128 : base + 256] = Wz[rows, cols]
            rz[:, base + 256 : base + 384] = Wxr[rows, cols]
    return _to_bf16(rz)


def _gate_consts(bxh, bhh, bxr, bhr):
    """[128, 512] packed const tile [Cu | Cr | Cz | Cxr]."""
    c = np.zeros((128, 512), np.float32)
    bz = np.asarray(bxh[:H], np.float32) + np.asarray(bhh[:H], np.float32)
    br = np.asarray(bxh[H:], np.float32) + np.asarray(bhh[H:], np.float32)
    c[:, 0:128] = _pack_vec(np.asarray(bhr, np.float32))
    c[:, 128:256] = _pack_vec(br)
    c[:, 256:384] = _pack_vec(bz)
    c[:, 384:512] = _pack_vec(np.asarray(bxr, np.float32))
    return _to_bf16(c)


def _pack_T(xt):
    """[B, 512] -> stationary image [128, 128]: out[p, 32k+b] = xt[b, 128k+p]."""
    a = np.asarray(xt, np.float32).T.reshape(KC, 128, B)  # [k, p, b]
    return a.transpose(1, 0, 2).reshape(128, KC * B)


def prepare_inputs(x, Wxh, bxh, Whh, bhh, Wxr, bxr, Whr, bhr, Wfc, bfc):
    """Host-side layout marshalling -> dict of 2D SBUF-image arrays."""
    h = {}
    # x stationary stream: [128, T*128], free = (t, 32k+b)
    xT = np.zeros((128, T * 128), np.float32)
    for t in range(T):
        xT[:, t * 128 : (t + 1) * 128] = _pack_T(x[:, t, :])
    h["xT"] = _to_bf16(xT)
    h["xrevT"] = _to_bf16(_pack_T(x[:, T - 1, :]))

    for l in range(2):
        h[f"wg{l}"] = _gate_mov_tiles(Whh[l, 0], Whr[l, 0])
        h[f"wx{l}"] = _x_mov_tiles(Wxh[l, 0], Wxr[l, 0])
        h[f"cg{l}"] = _gate_consts(bxh[l, 0], bhh[l, 0], bxr[l, 0], bhr[l, 0])
        # reverse cells: only x-projections + consts needed (h0 = 0 single step)
        h[f"wx{l}r"] = _x_mov_tiles(Wxh[l, 1], Wxr[l, 1])
        h[f"cg{l}r"] = _gate_consts(bxh[l, 1], bhh[l, 1], bxr[l, 1], bhr[l, 1])

    wfc = np.zeros((128, 8 * O), np.float32)
    for kk in range(8):
        wfc[:, kk * O : (kk + 1) * O] = np.asarray(Wfc, np.float32)[
            128 * kk : 128 * (kk + 1), :
        ]
    h["wfc"] = _to_bf16(wfc)
    h["bfcrep"] = np.repeat(np.asarray(bfc, np.float32)[None, :], B, axis=0)

    h["ideye"] = _to_bf16(np.eye(128, dtype=np.float32))
    h["ideyef"] = np.eye(128, dtype=np.float32)
    return h


def build_program(host, n_steps=T, split_waits=False):
    nc = bacc.Bacc(
        "TRN2", target_bir_lowering=False, debug=False, num_devices=NCORES
    )
    dram = {}
    for name, arr in host.items():
        dt = BF16 if arr.dtype == BFNP else F32
        dram[name] = nc.dram_tensor(name, list(arr.shape), dt, kind="ExternalInput")
    out_d = nc.dram_tensor("out", [B, O], F32, kind="ExternalOutput")

    with tile.TileContext(nc) as tc:
        _emit(tc, dram, out_d, n_steps)
    nc.compile()
    if split_waits:
        _split_multi_waits(nc)  # fallback, normally handled by Bacc.compile
    return nc


def _split_multi_waits(nc):
    """This container's walrus allows only ONE sync-wait per instruction
    (setupSyncWait: 'Too many sync wait commands'). Move extra waits onto
    preceding same-engine NoOps."""
    n_nop = 0
    for fn in nc.m.functions:
        for blk in fn.blocks:
            out = []
            changed = False
            for inst in blk.instructions:
                si = inst.sync_info
                if si is not None and si.on_wait and len(si.on_wait) > 1:
                    waits = list(si.on_wait)
                    for w in waits[:-1]:
                        n_nop += 1
                        out.append(
                            mybir.InstNoOp(
                                name=f"waitnop-{n_nop}",
                                engine=inst.engine,
                                ins=[],
                                outs=[],
                                sync_info=mybir.SyncInfo(on_wait=[w], on_update=[]),
                            )
                        )
                    inst = inst.__replace__(
                        sync_info=mybir.SyncInfo(
                            on_wait=[waits[-1]], on_update=list(si.on_update or [])
                        )
                    )
                    changed = True
                out.append(inst)
            if changed:
                blk.instructions = out


def _emit(tc, dram, out_d, n_steps):
    nc = tc.nc
    from contextlib import ExitStack

    ctx = ExitStack()
    consts = ctx.enter_context(tc.tile_pool(name="consts", bufs=1))
    hpool = ctx.enter_context(tc.tile_pool(name="h", bufs=3))
    chain = ctx.enter_context(tc.tile_pool(name="chain", bufs=3))
    psums = ctx.enter_context(tc.tile_pool(name="psum", bufs=3, space="PSUM"))

    sb = {}
    for name, d in dram.items():
        t = consts.tile(list(d.shape), d.dtype, name=f"sb_{name}", tag=name)
        nc.sync.dma_start(t[:], d.ap())
        sb[name] = t

    ideye, ideyef = sb["ideye"], sb["ideyef"]

    def new_h(l):
        return hpool.tile([128, 128], BF16, name=f"h{l}", tag=f"h{l}")

    def new_hT(l):
        return hpool.tile([128, 128], BF16, name=f"hT{l}", tag=f"hT{l}")

    h = {}
    hT = {}
    for l in (1, 2):
        h[l] = new_h(l)
        nc.vector.memset(h[l][:], 0.0)
        hT[l] = new_hT(l)
        nc.vector.memset(hT[l][:], 0.0)

    def mm_phase(tag, cg, xstat, xoff, wx, hstat, wg):
        """Emit one gate-bank accumulation. PSUM bank [u | r | z | xr].

        Tile only orders same-engine MMs via overlapping-write (WAW) deps,
        and psum start/stop state is tracked per partition range. Each 32-row
        column-group range is an independent group: the full-width id-MM
        (bias consts) starts it and overlap-dominates everything; the x-MMs
        [128:512] and gate-MMs [0:384] overlap each other in 128:384, so
        emission order is execution order; stop rides the last MM.

        xstat: stationary tile for the input stream, slices at xoff + 32k.
        hstat: stationary tile for the recurrence stream (or None).
        """
        L = psums.tile([128, 512], F32, name=f"L{tag}", tag="L")
        for gc in range(4):
            o = slice(32 * gc, 32 * gc + 32)
            tp = (0, 32 * gc)
            mms = [(L[o, 0:512], ideye[:, o], cg[:, :])]
            for k in range(KC):
                st = xstat[:, xoff + 32 * k : xoff + 32 * k + 32]
                mms.append(
                    (L[o, 128:512], st, wx[:, (k * 4 + gc) * 384 : (k * 4 + gc) * 384 + 384])
                )
            if hstat is not None:
                for k in range(KC):
                    st = hstat[:, 32 * k : 32 * k + 32]
                    mms.append(
                        (L[o, 0:384], st, wg[:, (k * 4 + gc) * 384 : (k * 4 + gc) * 384 + 384])
                    )
            for i, (o_ap, l_ap, r_ap) in enumerate(mms):
                nc.tensor.matmul(o_ap, l_ap, r_ap, start=(i == 0),
                                 stop=(i == len(mms) - 1), tile_position=tp)
        return L

    def chain_phase(tag, l, L, h_prev):
        """Gate nonlinearity + state update. Returns (h_new, hT_new).

        h' = z*h + (1-z)*g computed as p + q with p = sigm(z)*h off the
        critical path (runs during tanh) and c = 1-z = sigm(-z_pre) on ACT,
        so the post-tanh tail is just q = c*g -> h' = p + q.
        """
        rs = chain.tile([128, 128], F32, name=f"rs{tag}", tag=f"rs{tag}")
        nc.scalar.activation(rs[:], L[:, 128:256], AF.Sigmoid)
        cs = chain.tile([128, 128], F32, name=f"cs{tag}", tag=f"cs{tag}")
        nc.scalar.activation(cs[:], L[:, 256:384], AF.Sigmoid, scale=-1.0)
        if h_prev is not None:
            zs = chain.tile([128, 128], F32, name=f"zs{tag}", tag=f"zs{tag}")
            nc.scalar.activation(zs[:], L[:, 256:384], AF.Sigmoid)
        v0 = chain.tile([128, 128], F32, name=f"v0{tag}", tag=f"v0{tag}")
        nc.vector.tensor_mul(v0[:], L[:, 0:128], rs[:])
        v1 = chain.tile([128, 128], F32, name=f"v1{tag}", tag=f"v1{tag}")
        nc.vector.tensor_add(v1[:], v0[:], L[:, 384:512])
        hn = new_h(l) if l else chain.tile([128, 128], BF16, name=f"hr{tag}", tag=f"hr{tag}")
        if h_prev is not None:
            p = chain.tile([128, 128], F32, name=f"p{tag}", tag=f"p{tag}")
            nc.vector.tensor_mul(p[:], zs[:], h_prev[:])
        g = chain.tile([128, 128], F32, name=f"g{tag}", tag=f"g{tag}")
        nc.scalar.activation(g[:], v1[:], AF.Tanh)
        q = chain.tile([128, 128], F32, name=f"q{tag}", tag=f"q{tag}")
        nc.vector.tensor_mul(q[:], cs[:], g[:])
        if h_prev is not None:
            nc.vector.tensor_add(hn[:], p[:], q[:])
        else:
            # h0 = 0: h' = (1 - z) * g = q
            nc.vector.tensor_copy(hn[:], q[:])
        # full [128,128] transpose of the packed tile IS the hT layout:
        # Tp[p, 32k+b] = hn[32k+b, p] = h'[b, 128k+p]. bf16 end to end:
        # 1 cyc/row transpose and cheaper PSUM->SBUF copy.
        Tp = psums.tile([128, 128], BF16, name=f"T{l if l else tag}", tag="T")
        nc.tensor.transpose(Tp[:], hn[:], ideye[:])
        hTn = new_hT(l) if l else chain.tile([128, 128], BF16, name=f"hrT{tag}", tag=f"hrT{tag}")
        nc.scalar.copy(hTn[:], Tp[:])
        return hn, hTn

    # ---- forward recurrence, layers interleaved (l2 lags l1 by one step) ----
    for tau in range(n_steps + 1):
        s = tau - 1
        if 0 <= s:
            # layer 2 step s: input stream = xh2 from hT1(s); recurrence from hT2(s-1)
            L = mm_phase(
                "2", sb["cg1"], hT[1], 0, sb["wx1"],
                hT[2] if s > 0 else None, sb["wg1"],
            )
            h[2], hT[2] = chain_phase("2", 2, L, h[2] if s > 0 else None)
        if tau < n_steps:
            L = mm_phase(
                "1", sb["cg0"], sb["xT"], tau * 128, sb["wx0"],
                hT[1] if tau > 0 else None, sb["wg0"],
            )
            h[1], hT[1] = chain_phase("1", 1, L, h[1] if tau > 0 else None)

    # ---- reverse stream: one step per layer, h0 = 0 ----
    L = mm_phase("1", sb["cg0r"], sb["xrevT"], 0, sb["wx0r"], None, None)
    _, h1rT = chain_phase("r1", 0, L, None)
    L = mm_phase("2", sb["cg1r"], h1rT, 0, sb["wx1r"], None, None)
    _, h2rT = chain_phase("r2", 0, L, None)

    # ---- final FC: out = [h2_f ; h2_r] @ Wfc + bfc ----
    FCp = psums.tile([B, O], F32, name="FCp", tag="L")
    for kk in range(8):
        st = hT[2][:, 32 * kk : 32 * kk + 32] if kk < 4 else h2rT[:, 32 * (kk - 4) : 32 * (kk - 4) + 32]
        nc.tensor.matmul(
            FCp[:, :], st, sb["wfc"][:, kk * O : (kk + 1) * O],
            start=(kk == 0), stop=(kk == 7),
        )
    outsb = consts.tile([B, O], F32, name="outsb", tag="outsb")
    nc.vector.tensor_add(outsb[:], FCp[:], sb["bfcrep"][:])
    nc.sync.dma_start(out_d.ap(), outsb[:])
    ctx.close()


_CACHE = {}


def _run(host, trace=False, n_steps=T):
    key = ("prog", n_steps)
    if key not in _CACHE:
        _CACHE[key] = build_program(host, n_steps)
    nc = _CACHE[key]
    in_map = {k: np.ascontiguousarray(v) for k, v in host.items()}
    res = run_bass_kernel_spmd(
        nc, [in_map] * NCORES, core_ids=list(range(NCORES)), trace=trace
    )
    return res


def kernel(**inputs):
    host = prepare_inputs(**{k: np.asarray(v) for k, v in inputs.items()})
    res = _run(host, trace=False)
    return np.asarray(res.results[0]["out"], np.float32)



# revision 2
# speedup vs baseline: 1.4416x; 1.4416x over previous
"""Trainium2 Bass kernel for nn_BidirRecurrentModel (2-layer bidirectional GRU).

Strategy notes
--------------
The model output depends only on:
  * seq_f[-1] after layer-2 forward  -> needs the full 2x128-step forward recurrence
  * seq_r[0]  after layer-2 reverse  -> needs only ONE step per layer of the
    reverse cells (h0 = 0), fed by x[:, T-1, :]
so almost all work is the forward recurrence, which is latency-bound.

Layouts (hardcoded for B=32, T=128, I=H=O=512):
  packed [128, 128] tile: row 32*k + b <-> (chunk k of 4, batch b), col j = dim-in-chunk
  hT     [128, 128] bf16: hT[p, 32*k+b] = h[b, 128*k+p]  (stationary operand slices)
PSUM gate bank [128, 512] = [u | r | z | xr] accumulates:
  identity-matmul(bias consts) + x/xh-projection stream + h-recurrence stream,
  each as 4-way column-tiled matmuls (batch=32 per column group).
All matmul inputs bf16 (fp32 accumulate in PSUM); elementwise chain in fp32.
"""

import numpy as np

import concourse.bass as bass
import concourse.mybir as mybir
import concourse.tile as tile
from concourse import bacc
from concourse.bass_utils import run_bass_kernel_spmd

F32 = mybir.dt.float32
BF16 = mybir.dt.bfloat16
AF = mybir.ActivationFunctionType

B, T, I, H, O = 32, 128, 512, 512, 512
KC = 4          # 128-chunks over H (and I)
NCORES = 8

import ml_dtypes
BFNP = ml_dtypes.bfloat16


def _to_bf16(a):
    return np.asarray(a, np.float32).astype(BFNP)


def _pack_vec(v):
    """[512] -> packed replicated [128, 128]: out[32k+b, j] = v[128k+j]."""
    v = np.asarray(v, np.float32).reshape(KC, 128)
    out = np.repeat(v[:, None, :], B, axis=1)
    return out.reshape(128, 128)


def _gate_mov_tiles(Whh, Whr):
    """Moving tiles for the h-recurrence stream, PSUM bank layout [u|r|z|xr].

    Whh: [512, 1024] (z cols 0:512, r cols 512:1024), Whr: [512, 512].
    Returns [128, KC*4*384]: free index (k, gc, [u|r|z] x 128) -> bank 0:384.
    """
    Wz = Whh[:, :H]
    Wr = Whh[:, H:]
    out = np.zeros((128, KC * 4 * 384), np.float32)
    for k in range(KC):
        rows = slice(128 * k, 128 * (k + 1))
        for gc in range(4):
            cols = slice(128 * gc, 128 * (gc + 1))
            base = (k * 4 + gc) * 384
            out[:, base : base + 128] = Whr[rows, cols]
            out[:, base + 128 : base + 256] = Wr[rows, cols]
            out[:, base + 256 : base + 384] = Wz[rows, cols]
    return _to_bf16(out)


def _x_mov_tiles(Wxh, Wxr):
    """Moving tiles for the input-projection stream -> bank 128:512 [r|z|xr].

    Returns [128, KC*4*384]."""
    Wz = Wxh[:, :H]
    Wr = Wxh[:, H:]
    rz = np.zeros((128, KC * 4 * 384), np.float32)
    for k in range(KC):
        rows = slice(128 * k, 128 * (k + 1))
        for gc in range(4):
            cols = slice(128 * gc, 128 * (gc + 1))
            base = (k * 4 + gc) * 384
            rz[:, base : base + 128] = Wr[rows, cols]
            rz[:, base + 128 : base + 256] = Wz[rows, cols]
            rz[:, base + 256 : base + 384] = Wxr[rows, cols]
    return _to_bf16(rz)


def _gate_consts(bxh, bhh, bxr, bhr):
    """[128, 512] packed const tile [Cu | Cr | Cz | Cxr]."""
    c = np.zeros((128, 512), np.float32)
    bz = np.asarray(bxh[:H], np.float32) + np.asarray(bhh[:H], np.float32)
    br = np.asarray(bxh[H:], np.float32) + np.asarray(bhh[H:], np.float32)
    c[:, 0:128] = _pack_vec(np.asarray(bhr, np.float32))
    c[:, 128:256] = _pack_vec(br)
    c[:, 256:384] = _pack_vec(bz)
    c[:, 384:512] = _pack_vec(np.asarray(bxr, np.float32))
    return _to_bf16(c)


def _pack_T(xt):
    """[B, 512] -> stationary image [128, 128]: out[p, 32k+b] = xt[b, 128k+p]."""
    a = np.asarray(xt, np.float32).T.reshape(KC, 128, B)  # [k, p, b]
    return a.transpose(1, 0, 2).reshape(128, KC * B)


def prepare_inputs(x, Wxh, bxh, Whh, bhh, Wxr, bxr, Whr, bhr, Wfc, bfc):
    """Host-side layout marshalling -> dict of 2D SBUF-image arrays."""
    h = {}
    # x stationary stream: [128, T*128], free = (t, 32k+b)
    xT = np.zeros((128, T * 128), np.float32)
    for t in range(T):
        xT[:, t * 128 : (t + 1) * 128] = _pack_T(x[:, t, :])
    h["xT"] = _to_bf16(xT)
    h["xrevT"] = _to_bf16(_pack_T(x[:, T - 1, :]))

    for l in range(2):
        h[f"wg{l}"] = _gate_mov_tiles(Whh[l, 0], Whr[l, 0])
        h[f"wx{l}"] = _x_mov_tiles(Wxh[l, 0], Wxr[l, 0])
        h[f"cg{l}"] = _gate_consts(bxh[l, 0], bhh[l, 0], bxr[l, 0], bhr[l, 0])
        # reverse cells: only x-projections + consts needed (h0 = 0 single step)
        h[f"wx{l}r"] = _x_mov_tiles(Wxh[l, 1], Wxr[l, 1])
        h[f"cg{l}r"] = _gate_consts(bxh[l, 1], bhh[l, 1], bxr[l, 1], bhr[l, 1])

    wfc = np.zeros((128, 8 * O), np.float32)
    for kk in range(8):
        wfc[:, kk * O : (kk + 1) * O] = np.asarray(Wfc, np.float32)[
            128 * kk : 128 * (kk + 1), :
        ]
    h["wfc"] = _to_bf16(wfc)
    h["bfcrep"] = np.repeat(np.asarray(bfc, np.float32)[None, :], B, axis=0)

    h["ideye"] = _to_bf16(np.eye(128, dtype=np.float32))
    h["ideyef"] = np.eye(128, dtype=np.float32)
    return h


def build_program(host, n_steps=T, split_waits=False):
    nc = bacc.Bacc(
        "TRN2", target_bir_lowering=False, debug=False, num_devices=NCORES
    )
    dram = {}
    for name, arr in host.items():
        dt = BF16 if arr.dtype == BFNP else F32
        dram[name] = nc.dram_tensor(name, list(arr.shape), dt, kind="ExternalInput")
    out_d = nc.dram_tensor("out", [B, O], F32, kind="ExternalOutput")

    with tile.TileContext(nc) as tc:
        _emit(tc, dram, out_d, n_steps)
    nc.compile()
    if split_waits:
        _split_multi_waits(nc)  # fallback, normally handled by Bacc.compile
    return nc


def _split_multi_waits(nc):
    """This container's walrus allows only ONE sync-wait per instruction
    (setupSyncWait: 'Too many sync wait commands'). Move extra waits onto
    preceding same-engine NoOps."""
    n_nop = 0
    for fn in nc.m.functions:
        for blk in fn.blocks:
            out = []
            changed = False
            for inst in blk.instructions:
                si = inst.sync_info
                if si is not None and si.on_wait and len(si.on_wait) > 1:
                    waits = list(si.on_wait)
                    for w in waits[:-1]:
                        n_nop += 1
                        out.append(
                            mybir.InstNoOp(
                                name=f"waitnop-{n_nop}",
                                engine=inst.engine,
                                ins=[],
                                outs=[],
                                sync_info=mybir.SyncInfo(on_wait=[w], on_update=[]),
                            )
                        )
                    inst = inst.__replace__(
                        sync_info=mybir.SyncInfo(
                            on_wait=[waits[-1]], on_update=list(si.on_update or [])
                        )
                    )
                    changed = True
                out.append(inst)
            if changed:
                blk.instructions = out


def _emit(tc, dram, out_d, n_steps):
    nc = tc.nc
    from contextlib import ExitStack

    ctx = ExitStack()
    consts = ctx.enter_context(tc.tile_pool(name="consts", bufs=1))
    hpool = ctx.enter_context(tc.tile_pool(name="h", bufs=3))
    chain = ctx.enter_context(tc.tile_pool(name="chain", bufs=3))
    psums = ctx.enter_context(tc.tile_pool(name="psum", bufs=3, space="PSUM"))

    sb = {}
    for name, d in dram.items():
        t = consts.tile(list(d.shape), d.dtype, name=f"sb_{name}", tag=name)
        nc.sync.dma_start(t[:], d.ap())
        sb[name] = t

    ideye, ideyef = sb["ideye"], sb["ideyef"]

    def new_h(l):
        return hpool.tile([128, 128], BF16, name=f"h{l}", tag=f"h{l}")

    def new_hT(l):
        return hpool.tile([128, 128], BF16, name=f"hT{l}", tag=f"hT{l}")

    h = {}
    hT = {}
    for l in (1, 2):
        h[l] = new_h(l)
        nc.vector.memset(h[l][:], 0.0)
        hT[l] = new_hT(l)
        nc.vector.memset(hT[l][:], 0.0)

    def mm_phase(tag, cg, xstat, xoff, wx, hstat, wg):
        """Emit one gate-bank accumulation. PSUM bank [u | r | z | xr].

        Tile only orders same-engine MMs via overlapping-write (WAW) deps,
        and psum start/stop state is tracked per partition range. Each 32-row
        column-group range is an independent group: the full-width id-MM
        (bias consts) starts it and overlap-dominates everything; the x-MMs
        [128:512] and gate-MMs [0:384] overlap each other in 128:384, so
        emission order is execution order; stop rides the last MM.

        xstat: stationary tile for the input stream, slices at xoff + 32k.
        hstat: stationary tile for the recurrence stream (or None).
        """
        L = psums.tile([128, 512], F32, name=f"L{tag}", tag="L")
        per_gc = []
        for gc in range(4):
            o = slice(32 * gc, 32 * gc + 32)
            mms = [(L[o, 0:512], ideye[:, o], cg[:, :])]
            for k in range(KC):
                st = xstat[:, xoff + 32 * k : xoff + 32 * k + 32]
                mms.append(
                    (L[o, 128:512], st, wx[:, (k * 4 + gc) * 384 : (k * 4 + gc) * 384 + 384])
                )
            if hstat is not None:
                for k in range(KC):
                    st = hstat[:, 32 * k : 32 * k + 32]
                    mms.append(
                        (L[o, 0:384], st, wg[:, (k * 4 + gc) * 384 : (k * 4 + gc) * 384 + 384])
                    )
            per_gc.append(mms)
        # Interleave emission round-robin across the 4 col-strips: matmuls on
        # distinct 32-wide tile_position col groups execute concurrently in
        # the PE array, so strip-minor order keeps all 4 strips streaming
        # (grouped-by-strip order serializes them).
        n = len(per_gc[0])
        for i in range(n):
            for gc in range(4):
                o_ap, l_ap, r_ap = per_gc[gc][i]
                nc.tensor.matmul(o_ap, l_ap, r_ap, start=(i == 0),
                                 stop=(i == n - 1), tile_position=(0, 32 * gc))
        return L

    def chain_phase(tag, l, L, h_prev):
        """Gate nonlinearity + state update. Returns (h_new, hT_new).

        h' = z*h + (1-z)*g computed as p + q with p = sigm(z)*h off the
        critical path (runs during tanh) and c = 1-z = sigm(-z_pre) on ACT,
        so the post-tanh tail is just q = c*g -> h' = p + q.
        """
        rs = chain.tile([128, 128], F32, name=f"rs{tag}", tag=f"rs{tag}")
        nc.scalar.activation(rs[:], L[:, 128:256], AF.Sigmoid)
        cs = chain.tile([128, 128], F32, name=f"cs{tag}", tag=f"cs{tag}")
        nc.scalar.activation(cs[:], L[:, 256:384], AF.Sigmoid, scale=-1.0)
        if h_prev is not None:
            zs = chain.tile([128, 128], F32, name=f"zs{tag}", tag=f"zs{tag}")
            nc.scalar.activation(zs[:], L[:, 256:384], AF.Sigmoid)
        v0 = chain.tile([128, 128], F32, name=f"v0{tag}", tag=f"v0{tag}")
        nc.vector.tensor_mul(v0[:], L[:, 0:128], rs[:])
        v1 = chain.tile([128, 128], F32, name=f"v1{tag}", tag=f"v1{tag}")
        nc.vector.tensor_add(v1[:], v0[:], L[:, 384:512])
        hn = new_h(l) if l else chain.tile([128, 128], BF16, name=f"hr{tag}", tag=f"hr{tag}")
        if h_prev is not None:
            p = chain.tile([128, 128], F32, name=f"p{tag}", tag=f"p{tag}")
            nc.vector.tensor_mul(p[:], zs[:], h_prev[:])
        g = chain.tile([128, 128], F32, name=f"g{tag}", tag=f"g{tag}")
        nc.scalar.activation(g[:], v1[:], AF.Tanh)
        q = chain.tile([128, 128], F32, name=f"q{tag}", tag=f"q{tag}")
        nc.vector.tensor_mul(q[:], cs[:], g[:])
        if h_prev is not None:
            nc.vector.tensor_add(hn[:], p[:], q[:])
        else:
            # h0 = 0: h' = (1 - z) * g = q
            nc.vector.tensor_copy(hn[:], q[:])
        # full [128,128] transpose of the packed tile IS the hT layout:
        # Tp[p, 32k+b] = hn[32k+b, p] = h'[b, 128k+p]. bf16 end to end:
        # 1 cyc/row transpose and cheaper PSUM->SBUF copy.
        Tp = psums.tile([128, 128], BF16, name=f"T{l if l else tag}", tag="T")
        nc.tensor.transpose(Tp[:], hn[:], ideye[:])
        hTn = new_hT(l) if l else chain.tile([128, 128], BF16, name=f"hrT{tag}", tag=f"hrT{tag}")
        nc.scalar.copy(hTn[:], Tp[:])
        return hn, hTn

    # ---- forward recurrence, layers interleaved (l2 lags l1 by one step) ----
    for tau in range(n_steps + 1):
        s = tau - 1
        if 0 <= s:
            # layer 2 step s: input stream = xh2 from hT1(s); recurrence from hT2(s-1)
            L = mm_phase(
                "2", sb["cg1"], hT[1], 0, sb["wx1"],
                hT[2] if s > 0 else None, sb["wg1"],
            )
            h[2], hT[2] = chain_phase("2", 2, L, h[2] if s > 0 else None)
        if tau < n_steps:
            L = mm_phase(
                "1", sb["cg0"], sb["xT"], tau * 128, sb["wx0"],
                hT[1] if tau > 0 else None, sb["wg0"],
            )
            h[1], hT[1] = chain_phase("1", 1, L, h[1] if tau > 0 else None)

    # ---- reverse stream: one step per layer, h0 = 0 ----
    L = mm_phase("1", sb["cg0r"], sb["xrevT"], 0, sb["wx0r"], None, None)
    _, h1rT = chain_phase("r1", 0, L, None)
    L = mm_phase("2", sb["cg1r"], h1rT, 0, sb["wx1r"], None, None)
    _, h2rT = chain_phase("r2", 0, L, None)

    # ---- final FC: out = [h2_f ; h2_r] @ Wfc + bfc ----
    FCp = psums.tile([B, O], F32, name="FCp", tag="L")
    for kk in range(8):
        st = hT[2][:, 32 * kk : 32 * kk + 32] if kk < 4 else h2rT[:, 32 * (kk - 4) : 32 * (kk - 4) + 32]
        nc.tensor.matmul(
            FCp[:, :], st, sb["wfc"][:, kk * O : (kk + 1) * O],
            start=(kk == 0), stop=(kk == 7),
        )
    outsb = consts.tile([B, O], F32, name="outsb", tag="outsb")
    nc.vector.tensor_add(outsb[:], FCp[:], sb["bfcrep"][:])
    nc.sync.dma_start(out_d.ap(), outsb[:])
    ctx.close()


_CACHE = {}


def _run(host, trace=False, n_steps=T):
    key = ("prog", n_steps)
    if key not in _CACHE:
        _CACHE[key] = build_program(host, n_steps)
    nc = _CACHE[key]
    in_map = {k: np.ascontiguousarray(v) for k, v in host.items()}
    res = run_bass_kernel_spmd(
        nc, [in_map] * NCORES, core_ids=list(range(NCORES)), trace=trace
    )
    return res


def kernel(**inputs):
    host = prepare_inputs(**{k: np.asarray(v) for k, v in inputs.items()})
    res = _run(host, trace=False)
    return np.asarray(res.results[0]["out"], np.float32)



# revision 23
# speedup vs baseline: 2.2234x; 1.5423x over previous
"""Trainium2 Bass kernel for nn_BidirRecurrentModel (2-layer bidirectional GRU).

Strategy notes
--------------
The model output depends only on:
  * seq_f[-1] after layer-2 forward  -> needs the full 2x128-step forward recurrence
  * seq_r[0]  after layer-2 reverse  -> needs only ONE step per layer of the
    reverse cells (h0 = 0), fed by x[:, T-1, :]
so almost all work is the forward recurrence, which is latency-bound.

Layouts (hardcoded for B=32, T=128, I=H=O=512):
  packed [128, 128] tile: row 32*k + b <-> (chunk k of 4, batch b), col j = dim-in-chunk
  hT     [128, 128] bf16: hT[p, 32*k+b] = h[b, 128*k+p]  (stationary operand slices)
PSUM gate bank [128, 512] = [u | r | z | xr] accumulates:
  identity-matmul(bias consts) + x/xh-projection stream + h-recurrence stream,
  each as 4-way column-tiled matmuls (batch=32 per column group).
All matmul inputs bf16 (fp32 accumulate in PSUM); elementwise chain in fp32.
"""

import numpy as np

import concourse.bass as bass
import concourse.mybir as mybir
import concourse.tile as tile
from concourse import bacc
from concourse.bass_utils import run_bass_kernel_spmd

F32 = mybir.dt.float32
BF16 = mybir.dt.bfloat16
AF = mybir.ActivationFunctionType

B, T, I, H, O = 32, 128, 512, 512, 512
KC = 4          # 128-chunks over H (and I)
NCORES = 8

import ml_dtypes
BFNP = ml_dtypes.bfloat16


def _to_bf16(a):
    return np.asarray(a, np.float32).astype(BFNP)


def _pack_vec(v):
    """[512] -> packed replicated [128, 128]: out[32k+b, j] = v[128k+j]."""
    v = np.asarray(v, np.float32).reshape(KC, 128)
    out = np.repeat(v[:, None, :], B, axis=1)
    return out.reshape(128, 128)


def _gate_mov_tiles(Whh, Whr):
    """Moving tiles for the h-recurrence stream, PSUM bank layout [u|r|z|xr].

    Whh: [512, 1024] (z cols 0:512, r cols 512:1024), Whr: [512, 512].
    Returns [128, KC*4*384]: free index (k, gc, [u|r|z] x 128) -> bank 0:384.
    """
    Wz = Whh[:, :H]
    Wr = Whh[:, H:]
    out = np.zeros((128, KC * 4 * 384), np.float32)
    for k in range(KC):
        rows = slice(128 * k, 128 * (k + 1))
        for gc in range(4):
            cols = slice(128 * gc, 128 * (gc + 1))
            base = (k * 4 + gc) * 384
            out[:, base : base + 128] = Whr[rows, cols]
            out[:, base + 128 : base + 256] = Wr[rows, cols]
            out[:, base + 256 : base + 384] = Wz[rows, cols]
    return _to_bf16(out)


def _x_mov_tiles(Wxh, Wxr):
    """Moving tiles for the input-projection stream -> bank 128:512 [r|z|xr].

    Returns [128, KC*4*384]."""
    Wz = Wxh[:, :H]
    Wr = Wxh[:, H:]
    rz = np.zeros((128, KC * 4 * 384), np.float32)
    for k in range(KC):
        rows = slice(128 * k, 128 * (k + 1))
        for gc in range(4):
            cols = slice(128 * gc, 128 * (gc + 1))
            base = (k * 4 + gc) * 384
            rz[:, base : base + 128] = Wr[rows, cols]
            rz[:, base + 128 : base + 256] = Wz[rows, cols]
            rz[:, base + 256 : base + 384] = Wxr[rows, cols]
    return _to_bf16(rz)


def _gate_consts(bxh, bhh, bxr, bhr):
    """[128, 512] packed const tile [Cu | Cr | Cz | Cxr]."""
    c = np.zeros((128, 512), np.float32)
    bz = np.asarray(bxh[:H], np.float32) + np.asarray(bhh[:H], np.float32)
    br = np.asarray(bxh[H:], np.float32) + np.asarray(bhh[H:], np.float32)
    c[:, 0:128] = _pack_vec(np.asarray(bhr, np.float32))
    c[:, 128:256] = _pack_vec(br)
    c[:, 256:384] = _pack_vec(bz)
    c[:, 384:512] = _pack_vec(np.asarray(bxr, np.float32))
    return _to_bf16(c)


def _pack_T(xt):
    """[B, 512] -> stationary image [128, 128]: out[p, 32k+b] = xt[b, 128k+p]."""
    a = np.asarray(xt, np.float32).T.reshape(KC, 128, B)  # [k, p, b]
    return a.transpose(1, 0, 2).reshape(128, KC * B)


def prepare_inputs(x, Wxh, bxh, Whh, bhh, Wxr, bxr, Whr, bhr, Wfc, bfc):
    """Host-side layout marshalling -> dict of 2D SBUF-image arrays."""
    h = {}
    # x stationary stream: [128, T*128], free = (t, 32k+b)
    xT = np.zeros((128, T * 128), np.float32)
    for t in range(T):
        xT[:, t * 128 : (t + 1) * 128] = _pack_T(x[:, t, :])
    h["xT"] = _to_bf16(xT)
    h["xrevT"] = _to_bf16(_pack_T(x[:, T - 1, :]))

    for l in range(2):
        h[f"wg{l}"] = _gate_mov_tiles(Whh[l, 0], Whr[l, 0])
        h[f"wx{l}"] = _x_mov_tiles(Wxh[l, 0], Wxr[l, 0])
        h[f"cg{l}"] = _gate_consts(bxh[l, 0], bhh[l, 0], bxr[l, 0], bhr[l, 0])
        # reverse cells: only x-projections + consts needed (h0 = 0 single step)
        h[f"wx{l}r"] = _x_mov_tiles(Wxh[l, 1], Wxr[l, 1])
        h[f"cg{l}r"] = _gate_consts(bxh[l, 1], bhh[l, 1], bxr[l, 1], bhr[l, 1])

    wfc = np.zeros((128, 8 * O), np.float32)
    for kk in range(8):
        wfc[:, kk * O : (kk + 1) * O] = np.asarray(Wfc, np.float32)[
            128 * kk : 128 * (kk + 1), :
        ]
    h["wfc"] = _to_bf16(wfc)
    h["bfcrep"] = np.repeat(np.asarray(bfc, np.float32)[None, :], B, axis=0)

    h["ideye"] = _to_bf16(np.eye(128, dtype=np.float32))
    h["ideyef"] = np.eye(128, dtype=np.float32)
    return h


def build_program(host, n_steps=T, split_waits=False, reps=1):
    nc = bacc.Bacc(
        "TRN2", target_bir_lowering=False, debug=False, num_devices=NCORES
    )
    dram = {}
    for name, arr in host.items():
        dt = BF16 if arr.dtype == BFNP else F32
        dram[name] = nc.dram_tensor(name, list(arr.shape), dt, kind="ExternalInput")
    out_d = nc.dram_tensor("out", [B, O], F32, kind="ExternalOutput")

    with tile.TileContext(nc) as tc:
        for _ in range(reps):
            _emit(tc, dram, out_d, n_steps)
    nc.compile()
    if split_waits:
        _split_multi_waits(nc)  # fallback, normally handled by Bacc.compile
    return nc


def _split_multi_waits(nc):
    """This container's walrus allows only ONE sync-wait per instruction
    (setupSyncWait: 'Too many sync wait commands'). Move extra waits onto
    preceding same-engine NoOps."""
    n_nop = 0
    for fn in nc.m.functions:
        for blk in fn.blocks:
            out = []
            changed = False
            for inst in blk.instructions:
                si = inst.sync_info
                if si is not None and si.on_wait and len(si.on_wait) > 1:
                    waits = list(si.on_wait)
                    for w in waits[:-1]:
                        n_nop += 1
                        out.append(
                            mybir.InstNoOp(
                                name=f"waitnop-{n_nop}",
                                engine=inst.engine,
                                ins=[],
                                outs=[],
                                sync_info=mybir.SyncInfo(on_wait=[w], on_update=[]),
                            )
                        )
                    inst = inst.__replace__(
                        sync_info=mybir.SyncInfo(
                            on_wait=[waits[-1]], on_update=list(si.on_update or [])
                        )
                    )
                    changed = True
                out.append(inst)
            if changed:
                blk.instructions = out


def _emit(tc, dram, out_d, n_steps):
    nc = tc.nc
    from contextlib import ExitStack

    ctx = ExitStack()
    consts = ctx.enter_context(tc.tile_pool(name="consts", bufs=1))
    hpool = ctx.enter_context(tc.tile_pool(name="h", bufs=3))
    chain = ctx.enter_context(tc.tile_pool(name="chain", bufs=3))
    psums = ctx.enter_context(tc.tile_pool(name="psum", bufs=3, space="PSUM"))

    sb = {}
    for name, d in dram.items():
        t = consts.tile(list(d.shape), d.dtype, name=f"sb_{name}", tag=name)
        nc.sync.dma_start(t[:], d.ap())
        sb[name] = t

    ideye, ideyef = sb["ideye"], sb["ideyef"]

    def new_h(l):
        return hpool.tile([128, 128], BF16, name=f"h{l}", tag=f"h{l}")

    def new_hT(l):
        return hpool.tile([128, 128], BF16, name=f"hT{l}", tag=f"hT{l}")

    hT = {1: None, 2: None}

    def mm_phase(tag, cg, xstat, xoff, wx, hstat, wg):
        """Emit one gate-bank accumulation. PSUM bank [u | r | z | xr].

        Tile only orders same-engine MMs via overlapping-write (WAW) deps,
        and psum start/stop state is tracked per partition range. Each 32-row
        column-group range is an independent group: the full-width id-MM
        (bias consts) starts it and overlap-dominates everything; the x-MMs
        [128:512] and gate-MMs [0:384] overlap each other in 128:384, so
        emission order is execution order; stop rides the last MM.

        xstat: stationary tile for the input stream, slices at xoff + 32k.
        hstat: stationary tile for the recurrence stream (or None).
        """
        L = psums.tile([128, 512], F32, name=f"L{tag}", tag="L")
        per_gc = []
        for gc in range(4):
            o = slice(32 * gc, 32 * gc + 32)
            mms = [(L[o, 0:512], ideye[:, o], cg[:, :])]
            for k in range(KC):
                st = xstat[:, xoff + 32 * k : xoff + 32 * k + 32]
                mms.append(
                    (L[o, 128:512], st, wx[:, (k * 4 + gc) * 384 : (k * 4 + gc) * 384 + 384])
                )
            if hstat is not None:
                for k in range(KC):
                    st = hstat[:, 32 * k : 32 * k + 32]
                    mms.append(
                        (L[o, 0:384], st, wg[:, (k * 4 + gc) * 384 : (k * 4 + gc) * 384 + 384])
                    )
            per_gc.append(mms)
        # Interleave emission round-robin across the 4 col-strips: matmuls on
        # distinct 32-wide tile_position col groups execute concurrently in
        # the PE array, so strip-minor order keeps all 4 strips streaming
        # (grouped-by-strip order serializes them).
        n = len(per_gc[0])
        for i in range(n):
            for gc in range(4):
                o_ap, l_ap, r_ap = per_gc[gc][i]
                nc.tensor.matmul(o_ap, l_ap, r_ap, start=(i == 0),
                                 stop=(i == n - 1), tile_position=(0, 32 * gc),
                                 skip_group_check=True)
        return L

    def chain_core(tag, l, L, h_prev):
        """Gate nonlinearity + state update. Returns hn (packed layout).

        h' = z*h + (1-z)*g computed as p + q with p = sigm(z)*h off the
        critical path (runs during tanh) and c = 1-z = sigm(-z_pre) on ACT,
        so the post-tanh tail is just q = c*g -> h' = p + q. r and z
        sigmoids fuse into one [128,256] ACT op (contiguous PSUM banks).
        """
        rz = chain.tile([128, 256], F32, name=f"rz{tag}", tag=f"rz{tag}")
        nc.scalar.activation(rz[:], L[:, 128:384], AF.Sigmoid)
        cs = chain.tile([128, 128], F32, name=f"cs{tag}", tag=f"cs{tag}")
        nc.scalar.activation(cs[:], L[:, 256:384], AF.Sigmoid, scale=-1.0)
        v0 = chain.tile([128, 128], F32, name=f"v0{tag}", tag=f"v0{tag}")
        nc.vector.tensor_mul(v0[:], L[:, 0:128], rz[:, 0:128])
        v1 = chain.tile([128, 128], F32, name=f"v1{tag}", tag=f"v1{tag}")
        nc.vector.tensor_add(v1[:], v0[:], L[:, 384:512])
        hn = new_h(l) if l else chain.tile([128, 128], BF16, name=f"hr{tag}", tag=f"hr{tag}")
        if h_prev is not None:
            p = chain.tile([128, 128], F32, name=f"p{tag}", tag=f"p{tag}")
            nc.vector.tensor_mul(p[:], rz[:, 128:256], h_prev[:])
        g = chain.tile([128, 128], F32, name=f"g{tag}", tag=f"g{tag}")
        nc.scalar.activation(g[:], v1[:], AF.Tanh)
        q = chain.tile([128, 128], F32, name=f"q{tag}", tag=f"q{tag}")
        nc.vector.tensor_mul(q[:], cs[:], g[:])
        if h_prev is not None:
            nc.vector.tensor_add(hn[:], p[:], q[:])
        else:
            # h0 = 0: h' = (1 - z) * g = q
            nc.vector.tensor_copy(hn[:], q[:])
        return hn

    def hT_make(tag, l, hn):
        """Transpose packed hn -> stationary hT layout (PE transpose + copy).

        Emitted decoupled from chain_core so the PE hits it only after the
        chain had a full mm-phase's worth of time to finish (no PE stall).
        """
        Tp = psums.tile([128, 128], BF16, name=f"T{l if l else tag}", tag="T")
        nc.tensor.transpose(Tp[:], hn[:], ideye[:])
        hTn = new_hT(l) if l else chain.tile([128, 128], BF16, name=f"hrT{tag}", tag=f"hrT{tag}")
        nc.scalar.copy(hTn[:], Tp[:])
        return hTn

    # ---- forward recurrence, layers interleaved with lag 2 ----
    # Iter tau emits: T1(tau-1) | mm1(tau) | T2(tau-3) | mm2(tau-2).
    # Each transpose runs a full mm-phase after its chain was issued, so the
    # PE never stalls waiting on ACT/DVE; each mm-phase's h-MMs start after
    # the just-made hT (x/id-MMs first cover the PSUM->SBUF copy gap).
    hp = {1: None, 2: None}   # previous-step h (packed) per layer
    hn_pend = {1: None, 2: None}
    hT1_hist = {}             # step -> hT1 tile (last 2 generations alive)
    for tau in range(n_steps + 3):
        t1 = tau - 1
        if 0 <= t1 <= n_steps - 1:
            hT[1] = hT_make("1", 1, hn_pend[1])
            hT1_hist[t1] = hT[1]
        if tau < n_steps:
            L = mm_phase(
                "1", sb["cg0"], sb["xT"], tau * 128, sb["wx0"],
                hT[1] if tau > 0 else None, sb["wg0"],
            )
            hn = chain_core("1", 1, L, hp[1] if tau > 0 else None)
            hp[1], hn_pend[1] = hn, hn
        t2 = tau - 3
        if 0 <= t2 <= n_steps - 1:
            hT[2] = hT_make("2", 2, hn_pend[2])
        s = tau - 2
        if 0 <= s < n_steps:
            # layer 2 step s: input stream = xh2 from hT1(s); rec from hT2(s-1)
            L = mm_phase(
                "2", sb["cg1"], hT1_hist[s], 0, sb["wx1"],
                hT[2] if s > 0 else None, sb["wg1"],
            )
            hn = chain_core("2", 2, L, hp[2] if s > 0 else None)
            hp[2], hn_pend[2] = hn, hn

    # ---- reverse stream: one step per layer, h0 = 0 ----
    L = mm_phase("1", sb["cg0r"], sb["xrevT"], 0, sb["wx0r"], None, None)
    h1r = chain_core("r1", 0, L, None)
    h1rT = hT_make("r1", 0, h1r)
    L = mm_phase("2", sb["cg1r"], h1rT, 0, sb["wx1r"], None, None)
    h2r = chain_core("r2", 0, L, None)
    h2rT = hT_make("r2", 0, h2r)

    # ---- final FC: out = [h2_f ; h2_r] @ Wfc + bfc ----
    FCp = psums.tile([B, O], F32, name="FCp", tag="L")
    for kk in range(8):
        st = hT[2][:, 32 * kk : 32 * kk + 32] if kk < 4 else h2rT[:, 32 * (kk - 4) : 32 * (kk - 4) + 32]
        nc.tensor.matmul(
            FCp[:, :], st, sb["wfc"][:, kk * O : (kk + 1) * O],
            start=(kk == 0), stop=(kk == 7),
        )
    outsb = consts.tile([B, O], F32, name="outsb", tag="outsb")
    nc.vector.tensor_add(outsb[:], FCp[:], sb["bfcrep"][:])
    nc.sync.dma_start(out_d.ap(), outsb[:])
    ctx.close()


_CACHE = {}


def _run(host, trace=False, n_steps=T):
    key = ("prog", n_steps)
    if key not in _CACHE:
        _CACHE[key] = build_program(host, n_steps)
    nc = _CACHE[key]
    in_map = {k: np.ascontiguousarray(v) for k, v in host.items()}
    res = run_bass_kernel_spmd(
        nc, [in_map] * NCORES, core_ids=list(range(NCORES)), trace=trace
    )
    return res


def kernel(**inputs):
    host = prepare_inputs(**{k: np.asarray(v) for k, v in inputs.items()})
    res = _run(host, trace=False)
    return np.asarray(res.results[0]["out"], np.float32)



# revision 25
# speedup vs baseline: 2.7109x; 1.2193x over previous
"""Trainium2 Bass kernel for nn_BidirRecurrentModel (2-layer bidirectional GRU).

Strategy notes
--------------
The model output depends only on:
  * seq_f[-1] after layer-2 forward  -> needs the full 2x128-step forward recurrence
  * seq_r[0]  after layer-2 reverse  -> needs only ONE step per layer of the
    reverse cells (h0 = 0), fed by x[:, T-1, :]
so almost all work is the forward recurrence, which is latency-bound.

Layouts (hardcoded for B=32, T=128, I=H=O=512):
  packed [128, 128] tile: row 32*k + b <-> (chunk k of 4, batch b), col j = dim-in-chunk
  hT     [128, 128] bf16: hT[p, 32*k+b] = h[b, 128*k+p]  (stationary operand slices)
PSUM gate bank [128, 512] = [u | r | z | xr] accumulates:
  identity-matmul(bias consts) + x/xh-projection stream + h-recurrence stream,
  each as 4-way column-tiled matmuls (batch=32 per column group).
All matmul inputs bf16 (fp32 accumulate in PSUM); elementwise chain in fp32.
"""

import numpy as np

import concourse.bass as bass
import concourse.mybir as mybir
import concourse.tile as tile
from concourse import bacc
from concourse.bass_utils import run_bass_kernel_spmd

F32 = mybir.dt.float32
BF16 = mybir.dt.bfloat16
AF = mybir.ActivationFunctionType

B, T, I, H, O = 32, 128, 512, 512, 512
KC = 4          # 128-chunks over H (and I)
NCORES = 8

import ml_dtypes
BFNP = ml_dtypes.bfloat16


def _to_bf16(a):
    return np.asarray(a, np.float32).astype(BFNP)


def _pack_vec(v):
    """[512] -> packed replicated [128, 128]: out[32k+b, j] = v[128k+j]."""
    v = np.asarray(v, np.float32).reshape(KC, 128)
    out = np.repeat(v[:, None, :], B, axis=1)
    return out.reshape(128, 128)


def _gate_mov_tiles(Whh, Whr):
    """Moving tiles for the h-recurrence stream, PSUM bank layout [u|r|z|xr].

    Whh: [512, 1024] (z cols 0:512, r cols 512:1024), Whr: [512, 512].
    Returns [128, KC*4*384]: free index (k, gc, [u|r|z] x 128) -> bank 0:384.
    """
    Wz = Whh[:, :H]
    Wr = Whh[:, H:]
    out = np.zeros((128, KC * 4 * 384), np.float32)
    for k in range(KC):
        rows = slice(128 * k, 128 * (k + 1))
        for gc in range(4):
            cols = slice(128 * gc, 128 * (gc + 1))
            base = (k * 4 + gc) * 384
            out[:, base : base + 128] = Whr[rows, cols]
            out[:, base + 128 : base + 256] = Wr[rows, cols]
            out[:, base + 256 : base + 384] = Wz[rows, cols]
    return _to_bf16(out)


def _x_mov_tiles(Wxh, Wxr):
    """Moving tiles for the input-projection stream -> bank 128:512 [r|z|xr].

    Returns [128, KC*4*384]."""
    Wz = Wxh[:, :H]
    Wr = Wxh[:, H:]
    rz = np.zeros((128, KC * 4 * 384), np.float32)
    for k in range(KC):
        rows = slice(128 * k, 128 * (k + 1))
        for gc in range(4):
            cols = slice(128 * gc, 128 * (gc + 1))
            base = (k * 4 + gc) * 384
            rz[:, base : base + 128] = Wr[rows, cols]
            rz[:, base + 128 : base + 256] = Wz[rows, cols]
            rz[:, base + 256 : base + 384] = Wxr[rows, cols]
    return _to_bf16(rz)


def _gate_consts(bxh, bhh, bxr, bhr):
    """[128, 512] packed const tile [Cu | Cr | Cz | Cxr]."""
    c = np.zeros((128, 512), np.float32)
    bz = np.asarray(bxh[:H], np.float32) + np.asarray(bhh[:H], np.float32)
    br = np.asarray(bxh[H:], np.float32) + np.asarray(bhh[H:], np.float32)
    c[:, 0:128] = _pack_vec(np.asarray(bhr, np.float32))
    c[:, 128:256] = _pack_vec(br)
    c[:, 256:384] = _pack_vec(bz)
    c[:, 384:512] = _pack_vec(np.asarray(bxr, np.float32))
    return _to_bf16(c)


def _pack_T(xt):
    """[B, 512] -> stationary image [128, 128]: out[p, 32k+b] = xt[b, 128k+p]."""
    a = np.asarray(xt, np.float32).T.reshape(KC, 128, B)  # [k, p, b]
    return a.transpose(1, 0, 2).reshape(128, KC * B)


def prepare_inputs(x, Wxh, bxh, Whh, bhh, Wxr, bxr, Whr, bhr, Wfc, bfc):
    """Host-side layout marshalling -> dict of 2D SBUF-image arrays."""
    h = {}
    # x stationary stream: [128, T*128], free = (t, 32k+b)
    xT = np.zeros((128, T * 128), np.float32)
    for t in range(T):
        xT[:, t * 128 : (t + 1) * 128] = _pack_T(x[:, t, :])
    h["xT"] = _to_bf16(xT)
    h["xrevT"] = _to_bf16(_pack_T(x[:, T - 1, :]))

    for l in range(2):
        h[f"wg{l}"] = _gate_mov_tiles(Whh[l, 0], Whr[l, 0])
        h[f"wx{l}"] = _x_mov_tiles(Wxh[l, 0], Wxr[l, 0])
        h[f"cg{l}"] = _gate_consts(bxh[l, 0], bhh[l, 0], bxr[l, 0], bhr[l, 0])
        # reverse cells: only x-projections + consts needed (h0 = 0 single step)
        h[f"wx{l}r"] = _x_mov_tiles(Wxh[l, 1], Wxr[l, 1])
        h[f"cg{l}r"] = _gate_consts(bxh[l, 1], bhh[l, 1], bxr[l, 1], bhr[l, 1])

    wfc = np.zeros((128, 8 * O), np.float32)
    for kk in range(8):
        wfc[:, kk * O : (kk + 1) * O] = np.asarray(Wfc, np.float32)[
            128 * kk : 128 * (kk + 1), :
        ]
    h["wfc"] = _to_bf16(wfc)
    h["bfcrep"] = np.repeat(np.asarray(bfc, np.float32)[None, :], B, axis=0)

    h["ideye"] = _to_bf16(np.eye(128, dtype=np.float32))
    h["ideyef"] = np.eye(128, dtype=np.float32)
    return h


def build_program(host, n_steps=T, split_waits=False, reps=1):
    nc = bacc.Bacc(
        "TRN2", target_bir_lowering=False, debug=False, num_devices=NCORES
    )
    dram = {}
    for name, arr in host.items():
        dt = BF16 if arr.dtype == BFNP else F32
        dram[name] = nc.dram_tensor(name, list(arr.shape), dt, kind="ExternalInput")
    out_d = nc.dram_tensor("out", [B, O], F32, kind="ExternalOutput")

    with tile.TileContext(nc) as tc:
        for _ in range(reps):
            _emit(tc, dram, out_d, n_steps)
    nc.compile()
    if split_waits:
        _split_multi_waits(nc)  # fallback, normally handled by Bacc.compile
    return nc


def _split_multi_waits(nc):
    """This container's walrus allows only ONE sync-wait per instruction
    (setupSyncWait: 'Too many sync wait commands'). Move extra waits onto
    preceding same-engine NoOps."""
    n_nop = 0
    for fn in nc.m.functions:
        for blk in fn.blocks:
            out = []
            changed = False
            for inst in blk.instructions:
                si = inst.sync_info
                if si is not None and si.on_wait and len(si.on_wait) > 1:
                    waits = list(si.on_wait)
                    for w in waits[:-1]:
                        n_nop += 1
                        out.append(
                            mybir.InstNoOp(
                                name=f"waitnop-{n_nop}",
                                engine=inst.engine,
                                ins=[],
                                outs=[],
                                sync_info=mybir.SyncInfo(on_wait=[w], on_update=[]),
                            )
                        )
                    inst = inst.__replace__(
                        sync_info=mybir.SyncInfo(
                            on_wait=[waits[-1]], on_update=list(si.on_update or [])
                        )
                    )
                    changed = True
                out.append(inst)
            if changed:
                blk.instructions = out


def _emit(tc, dram, out_d, n_steps):
    nc = tc.nc
    from contextlib import ExitStack

    ctx = ExitStack()
    consts = ctx.enter_context(tc.tile_pool(name="consts", bufs=1))
    hpool = ctx.enter_context(tc.tile_pool(name="h", bufs=3))
    chain = ctx.enter_context(tc.tile_pool(name="chain", bufs=3))
    psums = ctx.enter_context(tc.tile_pool(name="psum", bufs=3, space="PSUM"))

    sb = {}
    for name, d in dram.items():
        t = consts.tile(list(d.shape), d.dtype, name=f"sb_{name}", tag=name)
        nc.sync.dma_start(t[:], d.ap())
        sb[name] = t

    ideye, ideyef = sb["ideye"], sb["ideyef"]

    def new_h(l):
        return hpool.tile([128, 128], BF16, name=f"h{l}", tag=f"h{l}")

    def new_hT(l):
        return hpool.tile([128, 128], BF16, name=f"hT{l}", tag=f"hT{l}")

    hT = {1: None, 2: None}

    def mm_phase(tag, cg, xstat, xoff, wx, hstat, wg):
        """Emit one gate-bank accumulation. PSUM bank [u | r | z | xr].

        Tile only orders same-engine MMs via overlapping-write (WAW) deps,
        and psum start/stop state is tracked per partition range. Each 32-row
        column-group range is an independent group: the full-width id-MM
        (bias consts) starts it and overlap-dominates everything; the x-MMs
        [128:512] and gate-MMs [0:384] overlap each other in 128:384, so
        emission order is execution order; stop rides the last MM.

        xstat: stationary tile for the input stream, slices at xoff + 32k.
        hstat: stationary tile for the recurrence stream (or None).
        """
        L = psums.tile([128, 512], F32, name=f"L{tag}", tag="L")
        per_gc = []
        for gc in range(4):
            o = slice(32 * gc, 32 * gc + 32)
            mms = [(L[o, 0:512], ideye[:, o], cg[:, :])]
            for k in range(KC):
                st = xstat[:, xoff + 32 * k : xoff + 32 * k + 32]
                mms.append(
                    (L[o, 128:512], st, wx[:, (k * 4 + gc) * 384 : (k * 4 + gc) * 384 + 384])
                )
            if hstat is not None:
                for k in range(KC):
                    st = hstat[:, 32 * k : 32 * k + 32]
                    mms.append(
                        (L[o, 0:384], st, wg[:, (k * 4 + gc) * 384 : (k * 4 + gc) * 384 + 384])
                    )
            per_gc.append(mms)
        # Interleave emission round-robin across the 4 col-strips: matmuls on
        # distinct 32-wide tile_position col groups execute concurrently in
        # the PE array, so strip-minor order keeps all 4 strips streaming
        # (grouped-by-strip order serializes them).
        n = len(per_gc[0])
        for i in range(n):
            for gc in range(4):
                o_ap, l_ap, r_ap = per_gc[gc][i]
                nc.tensor.matmul(o_ap, l_ap, r_ap, start=(i == 0),
                                 stop=(i == n - 1), tile_position=(0, 32 * gc),
                                 skip_group_check=True)
        return L

    def chain_core(tag, l, L, h_prev):
        """Gate nonlinearity + state update. Returns hn (packed layout).

        h' = z*h + (1-z)*g computed as p + q with p = sigm(z)*h off the
        critical path (runs during tanh) and c = 1-z = sigm(-z_pre) on ACT,
        so the post-tanh tail is just q = c*g -> h' = p + q. r and z
        sigmoids fuse into one [128,256] ACT op (contiguous PSUM banks).
        """
        rz = chain.tile([128, 256], F32, name=f"rz{tag}", tag=f"rz{tag}")
        nc.scalar.activation(rz[:], L[:, 128:384], AF.Sigmoid)
        cs = chain.tile([128, 128], F32, name=f"cs{tag}", tag=f"cs{tag}")
        nc.scalar.activation(cs[:], L[:, 256:384], AF.Sigmoid, scale=-1.0)
        v0 = chain.tile([128, 128], F32, name=f"v0{tag}", tag=f"v0{tag}")
        nc.vector.tensor_mul(v0[:], L[:, 0:128], rz[:, 0:128])
        v1 = chain.tile([128, 128], F32, name=f"v1{tag}", tag=f"v1{tag}")
        nc.vector.tensor_add(v1[:], v0[:], L[:, 384:512])
        hn = new_h(l) if l else chain.tile([128, 128], BF16, name=f"hr{tag}", tag=f"hr{tag}")
        if h_prev is not None:
            p = chain.tile([128, 128], F32, name=f"p{tag}", tag=f"p{tag}")
            nc.vector.tensor_mul(p[:], rz[:, 128:256], h_prev[:])
        g = chain.tile([128, 128], F32, name=f"g{tag}", tag=f"g{tag}")
        nc.scalar.activation(g[:], v1[:], AF.Tanh)
        q = chain.tile([128, 128], F32, name=f"q{tag}", tag=f"q{tag}")
        nc.vector.tensor_mul(q[:], cs[:], g[:])
        if h_prev is not None:
            nc.vector.tensor_add(hn[:], p[:], q[:])
        else:
            # h0 = 0: h' = (1 - z) * g = q
            nc.vector.tensor_copy(hn[:], q[:])
        return hn

    def hT_make(tag, l, hn):
        """Transpose packed hn -> stationary hT layout (PE transpose + copy).

        Emitted decoupled from chain_core so the PE hits it only after the
        chain had a full mm-phase's worth of time to finish (no PE stall).
        """
        Tp = psums.tile([128, 128], BF16, name=f"T{l if l else tag}", tag="T")
        nc.tensor.transpose(Tp[:], hn[:], ideye[:])
        hTn = new_hT(l) if l else chain.tile([128, 128], BF16, name=f"hrT{tag}", tag=f"hrT{tag}")
        nc.scalar.copy(hTn[:], Tp[:])
        return hTn

    # ---- forward recurrence, layers interleaved with lag 2 ----
    # Iter tau emits: T1(tau-1) | mm1(tau) | T2(tau-3) | mm2(tau-2).
    # Each transpose runs a full mm-phase after its chain was issued, so the
    # PE never stalls waiting on ACT/DVE; each mm-phase's h-MMs start after
    # the just-made hT (x/id-MMs first cover the PSUM->SBUF copy gap).
    rev = {}
    hp = {1: None, 2: None}   # previous-step h (packed) per layer
    hn_pend = {1: None, 2: None}
    hT1_hist = {}             # step -> hT1 tile (last 2 generations alive)
    for tau in range(n_steps + 3):
        t1 = tau - 1
        if 0 <= t1 <= n_steps - 1:
            hT[1] = hT_make("1", 1, hn_pend[1])
            hT1_hist[t1] = hT[1]
        if tau < n_steps:
            L = mm_phase(
                "1", sb["cg0"], sb["xT"], tau * 128, sb["wx0"],
                hT[1] if tau > 0 else None, sb["wg0"],
            )
            hn = chain_core("1", 1, L, hp[1] if tau > 0 else None)
            hp[1], hn_pend[1] = hn, hn
        t2 = tau - 3
        if 0 <= t2 <= n_steps - 1:
            hT[2] = hT_make("2", 2, hn_pend[2])
        # reverse stream (1 step/layer, h0=0) hoisted into the loop shadow:
        # independent of the forward scans, so its chains/transposes hide
        # under forward mm-phases instead of serializing after the loop.
        if tau == 2:
            Lr = mm_phase("1", sb["cg0r"], sb["xrevT"], 0, sb["wx0r"], None, None)
            rev["h1r"] = chain_core("r1", 0, Lr, None)
        if tau == 4:
            rev["h1rT"] = hT_make("r1", 0, rev["h1r"])
        if tau == 6:
            Lr = mm_phase("2", sb["cg1r"], rev["h1rT"], 0, sb["wx1r"], None, None)
            rev["h2r"] = chain_core("r2", 0, Lr, None)
        if tau == 8:
            rev["h2rT"] = hT_make("r2", 0, rev["h2r"])
        s = tau - 2
        if 0 <= s < n_steps:
            # layer 2 step s: input stream = xh2 from hT1(s); rec from hT2(s-1)
            L = mm_phase(
                "2", sb["cg1"], hT1_hist[s], 0, sb["wx1"],
                hT[2] if s > 0 else None, sb["wg1"],
            )
            hn = chain_core("2", 2, L, hp[2] if s > 0 else None)
            hp[2], hn_pend[2] = hn, hn


    # ---- final FC: out = [h2_f ; h2_r] @ Wfc + bfc ----
    FCp = psums.tile([B, O], F32, name="FCp", tag="L")
    for kk in range(8):
        st = hT[2][:, 32 * kk : 32 * kk + 32] if kk < 4 else rev["h2rT"][:, 32 * (kk - 4) : 32 * (kk - 4) + 32]
        nc.tensor.matmul(
            FCp[:, :], st, sb["wfc"][:, kk * O : (kk + 1) * O],
            start=(kk == 0), stop=(kk == 7),
        )
    outsb = consts.tile([B, O], F32, name="outsb", tag="outsb")
    nc.vector.tensor_add(outsb[:], FCp[:], sb["bfcrep"][:])
    nc.sync.dma_start(out_d.ap(), outsb[:])
    ctx.close()


_CACHE = {}


def _run(host, trace=False, n_steps=T):
    key = ("prog", n_steps)
    if key not in _CACHE:
        _CACHE[key] = build_program(host, n_steps)
    nc = _CACHE[key]
    in_map = {k: np.ascontiguousarray(v) for k, v in host.items()}
    res = run_bass_kernel_spmd(
        nc, [in_map] * NCORES, core_ids=list(range(NCORES)), trace=trace
    )
    return res


def kernel(**inputs):
    host = prepare_inputs(**{k: np.asarray(v) for k, v in inputs.items()})
    res = _run(host, trace=False)
    return np.asarray(res.results[0]["out"], np.float32)

